# revision 23
# baseline (speedup 1.0000x reference)
"""Causal self-attention block (qkv proj -> causal softmax attention -> out proj)
as a Bass/Tile SPMD kernel for 8 Trainium2 NeuronCores.

Sharding: data parallel over batch (B=2 -> 2 groups of 4 cores), tensor
parallel over heads within each group (12 heads -> 3 heads/core).  Each core:
  A. loads x[b], PE-transposes it to x^T in SBUF
  B. computes Q^T,K^T row-stacks and V (natural layout, augmented with a ones
     column per k-tile for the softmax denominator) for its 3 heads
  C. streaming causal attention per head without any P transposes:
       S^T tile = K_tile  Q^T          (matmul, lhsT = K^T slice)
       P^T = exp(scale*(S^T + mask))   (ScalarE, mask only on diagonal blocks)
       [O^T; sumexp] += [V|1]^T P^T    (matmul accumulate, 65-row psum)
       O^T *= 1/sumexp broadcast       (DVE recip + PE rank-1 broadcast)
     Attention emission is interleaved with phase B at q-slice granularity
     (slice j only needs x^T columns < 512(j+1)), so ScalarE exp work hides
     under phase B's PE work.  Heads 0,1 run interleaved; head 2 runs last so
     its compute covers the first two AllGathers.
  D. per-head AllGathers of O^T over the 4-core group, then the output
     projection for this core's quarter of the sequence (dynamic offset from
     partition id), PE-transposed back to natural layout.

All matmuls run in float32r (TF32-like, 1 cycle/row vs fp32's 4) with fp32
PSUM accumulation; measured matmul error ~1.5e-4 scale-relative.  Softmax
runs without max-subtraction (|scale*S| <= ~9 for this problem, exp is safe).

Matmul operands must share a base partition, so per-head 64-row Q^T/K^T
slices live in separate Q/K tensors at matching row offsets (heads 0,1 packed
in rows 0:64/64:128 of the Q and K stacks; head 2 in rows 0:64 of its own
pair, with 64 zero pad columns in the weight slice).
"""

import os
import sys

for _p in ("/opt/trn_rl_repo", "/root/.axon_site/_ro/trn_rl_repo"):
    if os.path.isdir(_p) and _p not in sys.path:
        sys.path.append(_p)

import numpy as np

B, T, C = 2, 4096, 768
H, DH = 12, 64
N_CORES = 8
G = 4                 # cores per batch group
HPC = 3               # heads per core
SC = HPC * DH         # 192: per-core width of each of Q/K/V
WC = 768              # weight cols: 128 each of Q01,K01,(Q2|pad),(K2|pad),V01,(V2|pad)
NQS = T // 512        # 8 q-slices of 512
NKT = T // 128        # 32 k-tiles of 128
QW = T // 4           # 1024: x processed in quarters
TQ = T // G           # 1024: per-core output rows
SCALE = 1.0 / np.sqrt(DH)
NEG = -1e30

_nc_cache = {}


def _ag_pieces(k):
    """Global O^T rows [128k,128k+128) as [(head, row, n)] pieces: the
    per-head AllGather h lays rank r's head-h rows at ag_out[h][64r:64r+64]."""
    out = []
    g = 128 * k
    while g < 128 * (k + 1):
        r, o192 = divmod(g, 192)
        i, o = divmod(o192, 64)
        out.append((i, 64 * r + o, 64))
        g += 64
    return out


def _build():
    import concourse.bass as bass
    import concourse.tile as tile
    import concourse.mybir as mybir
    from concourse import bacc
    from concourse.bass import ds

    f32 = mybir.dt.float32
    f32r = mybir.dt.float32r
    AF = mybir.ActivationFunctionType

    nc = bacc.Bacc(None, target_bir_lowering=False, debug=False, num_devices=N_CORES)

    xb = nc.dram_tensor("xb", [T, C], f32r, kind="ExternalInput")
    wqkv = nc.dram_tensor("wqkv", [C, WC], f32r, kind="ExternalInput")
    bqk = nc.dram_tensor("bqk", [768, 1], f32, kind="ExternalInput")
    wproj = nc.dram_tensor("wproj", [C, C], f32r, kind="ExternalInput")
    bproj = nc.dram_tensor("bproj", [C, 1], f32, kind="ExternalInput")
    masks = nc.dram_tensor("masks", [4, 128, 512], f32, kind="ExternalInput")
    ident = nc.dram_tensor("ident", [128, 128], f32r, kind="ExternalInput")
    onesd = nc.dram_tensor("onesd", [128, 32], f32r, kind="ExternalInput")

    ag_in = [nc.dram_tensor(f"ag_in{h}", [64, T], f32r) for h in range(HPC)]
    ag_out = [nc.dram_tensor(f"ag_out{h}", [4 * 64, T], f32r) for h in range(HPC)]
    yq = nc.dram_tensor("yq", [TQ, C], f32, kind="ExternalOutput")

    with tile.TileContext(nc) as tc:
        pid = nc.partition_id()
        qoff = (pid % G) * TQ

        with tc.tile_pool(name="const", bufs=1) as constp, \
             tc.tile_pool(name="wpj", bufs=1) as wpjp, \
             tc.tile_pool(name="psmm", bufs=6, space="PSUM") as ps_mm, \
             tc.tile_pool(name="psot", bufs=2, space="PSUM") as ps_ot:

            ident_sb = constp.tile([128, 128], f32r, name="ident_sb", tag="ident_sb")
            nc.sync.dma_start(ident_sb[:], ident[:])
            bqk_sb = constp.tile([128, 6], f32, name="bqk_sb", tag="bqk_sb")
            for m in range(6):
                nc.sync.dma_start(bqk_sb[:, m:m + 1], bqk[128 * m:128 * (m + 1), :])
            ones64 = constp.tile([1, 64], f32, name="ones64", tag="ones64")
            nc.vector.memset(ones64[:], 1.0)
            masks_sb = constp.tile([128, 4 * 512], f32, name="masks_sb", tag="masks_sb")
            bpj_sb = constp.tile([128, 6], f32, name="bpj_sb", tag="bpj_sb")

            with tc.tile_pool(name="qk", bufs=1) as qkp, \
                 tc.tile_pool(name="va", bufs=1) as vap, \
                 tc.tile_pool(name="pt", bufs=8) as ptp, \
                 tc.tile_pool(name="sm", bufs=3) as smp, \
                 tc.tile_pool(name="ost", bufs=3) as ostp:
                # qkt[0]=[Q0;Q1] qkt[1]=[K0;K1] qkt[2]=[Q2;pad] qkt[3]=[K2;pad]
                qkt = [qkp.tile([128, T], f32r, name=f"qkt{m}", tag=f"qkt{m}")
                       for m in range(4)]
                vaug = [vap.tile([128, NKT * 65], f32r, name=f"vaug{h}", tag=f"vaug{h}")
                        for h in range(HPC)]
                for h in range(HPC):
                    nc.sync.dma_start(vaug[h][:, 64:NKT * 65:65], onesd[:])

                hq = [(qkt[0], 0), (qkt[0], 64), (qkt[2], 0)]
                hk = [(qkt[1], 0), (qkt[1], 64), (qkt[3], 0)]

                def att(h, j):
                    """One (head, 512-wide q-slice) causal attention block.
                    S-matmul + exp are emitted ahead of the PV matmuls so the
                    scheduler can run S(k+1..) on PE while ACT computes
                    exp(k), instead of stalling PE on each PV's input."""
                    qt_t, qt_r = hq[h]
                    kt_t, kt_r = hk[h]
                    otps = ps_ot.tile([65, 512], f32, name="otps", tag="ot")
                    last = 4 * j + 3
                    pts = []
                    for k0 in range(4 * j + 4):
                        sps = ps_mm.tile([128, 512], f32, name="sps", tag="mm")
                        nc.tensor.matmul(
                            sps[:],
                            kt_t[kt_r:kt_r + 64, 128 * k0:128 * (k0 + 1)],
                            qt_t[qt_r:qt_r + 64, 512 * j:512 * (j + 1)],
                            start=True, stop=True)
                        pt = ptp.tile([128, 512], f32r, name="pt", tag="pt")
                        if k0 // 4 == j:
                            # diagonal: add the causal mask before exp
                            nc.vector.tensor_add(
                                pt[:], sps[:],
                                masks_sb[:, 512 * (k0 % 4):512 * (k0 % 4 + 1)])
                            nc.scalar.activation(pt[:], pt[:], AF.Exp, scale=SCALE)
                        else:
                            nc.scalar.activation(pt[:], sps[:], AF.Exp, scale=SCALE)
                        pts.append(pt)
                    for k0 in range(4 * j + 4):
                        nc.tensor.matmul(
                            otps[:], vaug[h][:, 65 * k0:65 * k0 + 65], pts[k0][:],
                            start=(k0 == 0), stop=(k0 == last))
                    rc = smp.tile([1, 512], f32, name="rc", tag="rc")
                    nc.vector.reciprocal(rc[:], otps[64:65, :])
                    bc = ps_mm.tile([64, 512], f32, name="bc", tag="mm")
                    nc.tensor.matmul(bc[:], ones64[:], rc[:], start=True, stop=True)
                    bcs = smp.tile([64, 512], f32, name="bcs", tag="bcs")
                    nc.vector.tensor_copy(bcs[:], bc[:])
                    ost = ostp.tile([64, 512], f32r, name="ost", tag="ost")
                    nc.vector.tensor_mul(ost[:], otps[0:64, :], bcs[:])
                    nc.sync.dma_start(
                        ag_in[h][:, 512 * j:512 * (j + 1)], ost[:])

                def allgather(h):
                    nc.gpsimd.collective_compute(
                        "AllGather", mybir.AluOpType.bypass,
                        replica_groups=[[0, 1, 2, 3], [4, 5, 6, 7]],
                        ins=[ag_in[h][:]], outs=[ag_out[h][:]])

                # ---- phase A+B quarters, interleaved with heads 0,1 ----------
                with tc.tile_pool(name="wq", bufs=1) as wqp, \
                     tc.tile_pool(name="xn", bufs=4) as xnp, \
                     tc.tile_pool(name="xt", bufs=1) as xtp:
                    wq = []
                    for k in range(6):
                        wt = wqp.tile([128, WC], f32r, name=f"wq{k}", tag=f"wq{k}")
                        nc.sync.dma_start(wt[:], wqkv[128 * k:128 * (k + 1), :])
                        wq.append(wt)
                    for d in range(4):
                        nc.sync.dma_start(masks_sb[:, 512 * d:512 * (d + 1)], masks[d])

                    for v in range(4):
                        xt = [xtp.tile([128, QW], f32r, name=f"xt{k}", tag=f"xt{k}")
                              for k in range(6)]
                        for tt in range(8):
                            xn = xnp.tile([128, C], f32r, name="xn", tag="xn")
                            nc.scalar.dma_start(
                                xn[:], xb[QW * v + 128 * tt:QW * v + 128 * (tt + 1), :])
                            for k in range(6):
                                tp = ps_mm.tile([128, 128], f32r, name="tp", tag="mm")
                                nc.tensor.transpose(
                                    tp[:], xn[:, 128 * k:128 * (k + 1)], ident_sb[:])
                                nc.vector.tensor_copy(
                                    xt[k][:, 128 * tt:128 * (tt + 1)], tp[:])
                        # Q^T/K^T/V^T rows (6 m-tiles of 128); V^T m-tiles
                        # (4,5) are PE-transposed into natural-layout vaug
                        for n2 in range(2):
                            ng = 2 * v + n2
                            for m in range(6):
                                ps = ps_mm.tile([128, 512], f32, name="psb", tag="mm")
                                for k in range(6):
                                    nc.tensor.matmul(
                                        ps[:], wq[k][:, 128 * m:128 * (m + 1)],
                                        xt[k][:, 512 * n2:512 * (n2 + 1)],
                                        start=(k == 0), stop=(k == 5))
                                if m < 4:
                                    nc.vector.tensor_scalar_add(
                                        qkt[m][:, 512 * ng:512 * (ng + 1)], ps[:],
                                        bqk_sb[:, m:m + 1])
                                    continue
                                vt = ptp.tile([128, 512], f32r, name="vt", tag="pt")
                                nc.vector.tensor_scalar_add(
                                    vt[:], ps[:], bqk_sb[:, m:m + 1])
                                nheads = 2 if m == 4 else 1
                                for sub in range(4):
                                    gk = 4 * ng + sub
                                    tpv = ps_mm.tile([128, 128], f32r,
                                                     name="tpv", tag="mm")
                                    nc.tensor.transpose(
                                        tpv[:], vt[:, 128 * sub:128 * (sub + 1)],
                                        ident_sb[:])
                                    for i in range(nheads):
                                        hh = 2 * (m - 4) + i
                                        nc.vector.tensor_copy(
                                            vaug[hh][:, 65 * gk:65 * gk + 64],
                                            tpv[:, 64 * i:64 * (i + 1)])
                        # attention slices that just became ready (head 0)
                        att(0, 2 * v)
                        att(0, 2 * v + 1)
                        if v == 3:
                            allgather(0)

                # proj weights load under heads 1,2 compute
                for m in range(6):
                    nc.sync.dma_start(bpj_sb[:, m:m + 1],
                                      bproj[128 * m:128 * (m + 1), :])
                wpj = []
                for k in range(6):
                    wt = wpjp.tile([128, C], f32r, name=f"wpj{k}", tag=f"wpj{k}")
                    nc.sync.dma_start(wt[:], wproj[128 * k:128 * (k + 1), :])
                    wpj.append(wt)

                # ---- heads 1,2 (their compute covers the AllGathers) --------
                for j in range(NQS):
                    att(1, j)
                allgather(1)
                for j in range(NQS):
                    att(2, j)
                allgather(2)

            # ---------------- phase D: output projection ---------------------
            with tc.tile_pool(name="prhs", bufs=2) as prp, \
                 tc.tile_pool(name="pst", bufs=1) as pstp, \
                 tc.tile_pool(name="yst", bufs=2) as ystp:
                pT = [pstp.tile([128, TQ], f32r, name=f"pT{m}", tag=f"pT{m}")
                      for m in range(6)]
                for n2 in range(2):
                    rhs = []
                    for k in range(6):
                        rt = prp.tile([128, 512], f32r, name=f"rhs{k}", tag=f"rhs{k}")
                        row = 0
                        for src, srow, n in _ag_pieces(k):
                            nc.sync.dma_start(
                                rt[row:row + n, :],
                                ag_out[src][srow:srow + n,
                                            ds(qoff + 512 * n2, 512)])
                            row += n
                        rhs.append(rt)
                    for m in range(6):
                        ps = ps_mm.tile([128, 512], f32, name="psp", tag="mm")
                        for k in range(6):
                            nc.tensor.matmul(
                                ps[:], wpj[k][:, 128 * m:128 * (m + 1)], rhs[k][:],
                                start=(k == 0), stop=(k == 5))
                        nc.vector.tensor_scalar_add(
                            pT[m][:, 512 * n2:512 * (n2 + 1)], ps[:],
                            bpj_sb[:, m:m + 1])
                for t in range(8):
                    yt = ystp.tile([128, C], f32, name="yt", tag="yt")
                    for m in range(6):
                        tp = ps_mm.tile([128, 128], f32r, name="tpy", tag="mm")
                        nc.tensor.transpose(
                            tp[:], pT[m][:, 128 * t:128 * (t + 1)], ident_sb[:])
                        nc.vector.tensor_copy(yt[:, 128 * m:128 * (m + 1)], tp[:])
                    nc.sync.dma_start(yq[128 * t:128 * (t + 1), :], yt[:])

    nc.finalize()
    return nc


def _get_nc():
    if "nc" not in _nc_cache:
        _nc_cache["nc"] = _build()
    return _nc_cache["nc"]


def _host_inputs(x, W_qkv, b_qkv, W_proj, b_proj):
    x = np.ascontiguousarray(np.asarray(x, dtype=np.float32))
    W_qkv = np.asarray(W_qkv, dtype=np.float32)
    b_qkv = np.asarray(b_qkv, dtype=np.float32)
    W_proj = np.ascontiguousarray(np.asarray(W_proj, dtype=np.float32))
    b_proj = np.asarray(b_proj, dtype=np.float32)

    kk = np.arange(128)[:, None]
    qq = np.arange(512)[None, :]
    masks = np.empty((4, 128, 512), np.float32)
    for d in range(4):
        masks[d] = np.where(128 * d + kk <= qq, 0.0, NEG)
    ident = np.eye(128, dtype=np.float32)

    zpad = np.zeros((C, 64), np.float32)
    zb = np.zeros((64,), np.float32)

    in_maps = []
    for c in range(N_CORES):
        b, g = divmod(c, G)
        heads = [HPC * g + i for i in range(HPC)]
        qc = [W_qkv[:, 64 * h:64 * (h + 1)] for h in heads]
        kc = [W_qkv[:, C + 64 * h:C + 64 * (h + 1)] for h in heads]
        vc = [W_qkv[:, 2 * C + 64 * h:2 * C + 64 * (h + 1)] for h in heads]
        # cols: [Q0 Q1 | K0 K1 | Q2 pad | K2 pad | V0 V1 | V2 pad]
        wqkv_c = np.ascontiguousarray(np.concatenate(
            [qc[0], qc[1], kc[0], kc[1], qc[2], zpad, kc[2], zpad,
             vc[0], vc[1], vc[2], zpad], axis=1))
        bqh = [b_qkv[64 * h:64 * (h + 1)] for h in heads]
        bkh = [b_qkv[C + 64 * h:C + 64 * (h + 1)] for h in heads]
        bvh = [b_qkv[2 * C + 64 * h:2 * C + 64 * (h + 1)] for h in heads]
        bqk_c = np.concatenate(
            [bqh[0], bqh[1], bkh[0], bkh[1], bqh[2], zb, bkh[2], zb,
             bvh[0], bvh[1], bvh[2], zb])
        in_maps.append({
            "xb": x[b],
            "wqkv": wqkv_c,
            "bqk": np.ascontiguousarray(bqk_c.reshape(C, 1)),
            "wproj": W_proj,
            "bproj": np.ascontiguousarray(b_proj.reshape(C, 1)),
            "masks": masks,
            "ident": ident,
            "onesd": np.ones((128, 32), np.float32),
        })
    return in_maps


def kernel(x, W_qkv, b_qkv, W_proj, b_proj, _trace=False):
    import time
    from concourse.bass_utils import run_bass_kernel_spmd

    nc = _get_nc()
    in_maps = _host_inputs(x, W_qkv, b_qkv, W_proj, b_proj)
    last_err = None
    for attempt in range(3):
        try:
            res = run_bass_kernel_spmd(nc, in_maps, list(range(N_CORES)),
                                       trace=_trace)
            break
        except Exception as e:  # the axon terminal is transiently flaky
            last_err = e
            time.sleep(20)
    else:
        raise last_err
    y = np.empty((B, T, C), np.float32)
    for c in range(N_CORES):
        b, g = divmod(c, G)
        y[b, TQ * g:TQ * (g + 1), :] = res.results[c]["yq"]
    if _trace:
        kernel.last_results = res
    return y


# revision 24
# speedup vs baseline: 1.0069x; 1.0069x over previous
"""Causal self-attention block (qkv proj -> causal softmax attention -> out proj)
as a Bass/Tile SPMD kernel for 8 Trainium2 NeuronCores.

Sharding: data parallel over batch (B=2 -> 2 groups of 4 cores), tensor
parallel over heads within each group (12 heads -> 3 heads/core).  Each core:
  A. loads x[b], PE-transposes it to x^T in SBUF
  B. computes Q^T,K^T row-stacks and V (natural layout, augmented with a ones
     column per k-tile for the softmax denominator) for its 3 heads
  C. streaming causal attention per head without any P transposes:
       S^T tile = K_tile  Q^T          (matmul, lhsT = K^T slice)
       P^T = exp(scale*(S^T + mask))   (ScalarE, mask only on diagonal blocks)
       [O^T; sumexp] += [V|1]^T P^T    (matmul accumulate, 65-row psum)
       O^T *= 1/sumexp broadcast       (DVE recip + PE rank-1 broadcast)
     Attention emission is interleaved with phase B at q-slice granularity
     (slice j only needs x^T columns < 512(j+1)), so ScalarE exp work hides
     under phase B's PE work.  Heads 0,1 run interleaved; head 2 runs last so
     its compute covers the first two AllGathers.
  D. per-head AllGathers of O^T over the 4-core group, then the output
     projection for this core's quarter of the sequence (dynamic offset from
     partition id), PE-transposed back to natural layout.

All matmuls run in float32r (TF32-like, 1 cycle/row vs fp32's 4) with fp32
PSUM accumulation; measured matmul error ~1.5e-4 scale-relative.  Softmax
runs without max-subtraction (|scale*S| <= ~9 for this problem, exp is safe).

Matmul operands must share a base partition, so per-head 64-row Q^T/K^T
slices live in separate Q/K tensors at matching row offsets (heads 0,1 packed
in rows 0:64/64:128 of the Q and K stacks; head 2 in rows 0:64 of its own
pair, with 64 zero pad columns in the weight slice).
"""

import os
import sys

for _p in ("/opt/trn_rl_repo", "/root/.axon_site/_ro/trn_rl_repo"):
    if os.path.isdir(_p) and _p not in sys.path:
        sys.path.append(_p)

import numpy as np

B, T, C = 2, 4096, 768
H, DH = 12, 64
N_CORES = 8
G = 4                 # cores per batch group
HPC = 3               # heads per core
SC = HPC * DH         # 192: per-core width of each of Q/K/V
WC = 768              # weight cols: 128 each of Q01,K01,(Q2|pad),(K2|pad),V01,(V2|pad)
NQS = T // 512        # 8 q-slices of 512
NKT = T // 128        # 32 k-tiles of 128
QW = T // 4           # 1024: x processed in quarters
TQ = T // G           # 1024: per-core output rows
SCALE = 1.0 / np.sqrt(DH)
NEG = -1e30

_nc_cache = {}


def _ag_pieces(k):
    """Global O^T rows [128k,128k+128) as [(head, row, n)] pieces: the
    per-head AllGather h lays rank r's head-h rows at ag_out[h][64r:64r+64]."""
    out = []
    g = 128 * k
    while g < 128 * (k + 1):
        r, o192 = divmod(g, 192)
        i, o = divmod(o192, 64)
        out.append((i, 64 * r + o, 64))
        g += 64
    return out


def _build():
    import concourse.bass as bass
    import concourse.tile as tile
    import concourse.mybir as mybir
    from concourse import bacc
    from concourse.bass import ds

    f32 = mybir.dt.float32
    f32r = mybir.dt.float32r
    AF = mybir.ActivationFunctionType

    nc = bacc.Bacc(None, target_bir_lowering=False, debug=False, num_devices=N_CORES)

    xb = nc.dram_tensor("xb", [T, C], f32r, kind="ExternalInput")
    wqkv = nc.dram_tensor("wqkv", [C, WC], f32r, kind="ExternalInput")
    bqk = nc.dram_tensor("bqk", [768, 1], f32, kind="ExternalInput")
    wproj = nc.dram_tensor("wproj", [C, C], f32r, kind="ExternalInput")
    bproj = nc.dram_tensor("bproj", [C, 1], f32, kind="ExternalInput")
    masks = nc.dram_tensor("masks", [4, 128, 512], f32, kind="ExternalInput")
    ident = nc.dram_tensor("ident", [128, 128], f32r, kind="ExternalInput")
    onesd = nc.dram_tensor("onesd", [128, 32], f32r, kind="ExternalInput")
    zerosd = nc.dram_tensor("zerosd", [128, 384], f32r, kind="ExternalInput")

    ag_in = [nc.dram_tensor(f"ag_in{h}", [64, T], f32r) for h in range(HPC)]
    ag_out = [nc.dram_tensor(f"ag_out{h}", [4 * 64, T], f32r) for h in range(HPC)]
    yq = nc.dram_tensor("yq", [TQ, C], f32, kind="ExternalOutput")

    with tile.TileContext(nc) as tc:
        pid = nc.partition_id()
        qoff = (pid % G) * TQ

        with tc.tile_pool(name="const", bufs=1) as constp, \
             tc.tile_pool(name="wpj", bufs=1) as wpjp, \
             tc.tile_pool(name="psmm", bufs=6, space="PSUM") as ps_mm, \
             tc.tile_pool(name="psot", bufs=2, space="PSUM") as ps_ot:

            ident_sb = constp.tile([128, 128], f32r, name="ident_sb", tag="ident_sb")
            nc.sync.dma_start(ident_sb[:], ident[:])
            bqk_sb = constp.tile([128, 6], f32, name="bqk_sb", tag="bqk_sb")
            for m in range(6):
                nc.sync.dma_start(bqk_sb[:, m:m + 1], bqk[128 * m:128 * (m + 1), :])
            ones64 = constp.tile([1, 64], f32, name="ones64", tag="ones64")
            nc.vector.memset(ones64[:], 1.0)
            masks_sb = constp.tile([128, 4 * 512], f32, name="masks_sb", tag="masks_sb")
            bpj_sb = constp.tile([128, 6], f32, name="bpj_sb", tag="bpj_sb")
            zeros_sb = constp.tile([128, 384], f32r, name="zeros_sb", tag="zeros_sb")
            nc.sync.dma_start(zeros_sb[:], zerosd[:])

            with tc.tile_pool(name="qk", bufs=1) as qkp, \
                 tc.tile_pool(name="va", bufs=1) as vap, \
                 tc.tile_pool(name="pt", bufs=8) as ptp, \
                 tc.tile_pool(name="sm", bufs=3) as smp, \
                 tc.tile_pool(name="ost", bufs=3) as ostp:
                # qkt[0]=[Q0;Q1] qkt[1]=[K0;K1] qkt[2]=[Q2;pad] qkt[3]=[K2;pad]
                qkt = [qkp.tile([128, T], f32r, name=f"qkt{m}", tag=f"qkt{m}")
                       for m in range(4)]
                vaug = [vap.tile([128, NKT * 65], f32r, name=f"vaug{h}", tag=f"vaug{h}")
                        for h in range(HPC)]
                for h in range(HPC):
                    nc.sync.dma_start(vaug[h][:, 64:NKT * 65:65], onesd[:])

                hq = [(qkt[0], 0), (qkt[0], 64), (qkt[2], 0)]
                hk = [(qkt[1], 0), (qkt[1], 64), (qkt[3], 0)]

                def att(h, j):
                    """One (head, 512-wide q-slice) causal attention block.
                    S-matmul + exp are emitted ahead of the PV matmuls so the
                    scheduler can run S(k+1..) on PE while ACT computes
                    exp(k), instead of stalling PE on each PV's input."""
                    qt_t, qt_r = hq[h]
                    kt_t, kt_r = hk[h]
                    otps = ps_ot.tile([65, 512], f32, name="otps", tag="ot")
                    last = 4 * j + 3
                    pts = []
                    for k0 in range(4 * j + 4):
                        sps = ps_mm.tile([128, 512], f32, name="sps", tag="mm")
                        nc.tensor.matmul(
                            sps[:],
                            kt_t[kt_r:kt_r + 64, 128 * k0:128 * (k0 + 1)],
                            qt_t[qt_r:qt_r + 64, 512 * j:512 * (j + 1)],
                            start=True, stop=True)
                        pt = ptp.tile([128, 512], f32r, name="pt", tag="pt")
                        if k0 // 4 == j:
                            # diagonal: cols < d0 are fully masked -> 0; add
                            # the causal mask and exp only on cols >= d0
                            d0 = 128 * (k0 % 4)
                            if d0 > 0:
                                nc.vector.tensor_copy(pt[:, 0:d0],
                                                      zeros_sb[:, 0:d0])
                            nc.vector.tensor_add(
                                pt[:, d0:512], sps[:, d0:512],
                                masks_sb[:, 512 * (k0 % 4) + d0:512 * (k0 % 4 + 1)])
                            nc.scalar.activation(
                                pt[:, d0:512], pt[:, d0:512], AF.Exp, scale=SCALE)
                        else:
                            nc.scalar.activation(pt[:], sps[:], AF.Exp, scale=SCALE)
                        pts.append(pt)
                    for k0 in range(4 * j + 4):
                        nc.tensor.matmul(
                            otps[:], vaug[h][:, 65 * k0:65 * k0 + 65], pts[k0][:],
                            start=(k0 == 0), stop=(k0 == last))
                    rc = smp.tile([1, 512], f32, name="rc", tag="rc")
                    nc.vector.reciprocal(rc[:], otps[64:65, :])
                    bc = ps_mm.tile([64, 512], f32, name="bc", tag="mm")
                    nc.tensor.matmul(bc[:], ones64[:], rc[:], start=True, stop=True)
                    bcs = smp.tile([64, 512], f32, name="bcs", tag="bcs")
                    nc.vector.tensor_copy(bcs[:], bc[:])
                    ost = ostp.tile([64, 512], f32r, name="ost", tag="ost")
                    nc.vector.tensor_mul(ost[:], otps[0:64, :], bcs[:])
                    nc.sync.dma_start(
                        ag_in[h][:, 512 * j:512 * (j + 1)], ost[:])

                def allgather(h):
                    nc.gpsimd.collective_compute(
                        "AllGather", mybir.AluOpType.bypass,
                        replica_groups=[[0, 1, 2, 3], [4, 5, 6, 7]],
                        ins=[ag_in[h][:]], outs=[ag_out[h][:]])

                # ---- phase A+B quarters, interleaved with heads 0,1 ----------
                with tc.tile_pool(name="wq", bufs=1) as wqp, \
                     tc.tile_pool(name="xn", bufs=4) as xnp, \
                     tc.tile_pool(name="xt", bufs=1) as xtp:
                    wq = []
                    for k in range(6):
                        wt = wqp.tile([128, WC], f32r, name=f"wq{k}", tag=f"wq{k}")
                        nc.sync.dma_start(wt[:], wqkv[128 * k:128 * (k + 1), :])
                        wq.append(wt)
                    for d in range(4):
                        nc.sync.dma_start(masks_sb[:, 512 * d:512 * (d + 1)], masks[d])

                    for v in range(4):
                        xt = [xtp.tile([128, QW], f32r, name=f"xt{k}", tag=f"xt{k}")
                              for k in range(6)]
                        for tt in range(8):
                            xn = xnp.tile([128, C], f32r, name="xn", tag="xn")
                            nc.scalar.dma_start(
                                xn[:], xb[QW * v + 128 * tt:QW * v + 128 * (tt + 1), :])
                            for k in range(6):
                                tp = ps_mm.tile([128, 128], f32r, name="tp", tag="mm")
                                nc.tensor.transpose(
                                    tp[:], xn[:, 128 * k:128 * (k + 1)], ident_sb[:])
                                nc.vector.tensor_copy(
                                    xt[k][:, 128 * tt:128 * (tt + 1)], tp[:])
                        # Q^T/K^T/V^T rows (6 m-tiles of 128); V^T m-tiles
                        # (4,5) are PE-transposed into natural-layout vaug
                        for n2 in range(2):
                            ng = 2 * v + n2
                            for m in range(6):
                                ps = ps_mm.tile([128, 512], f32, name="psb", tag="mm")
                                for k in range(6):
                                    nc.tensor.matmul(
                                        ps[:], wq[k][:, 128 * m:128 * (m + 1)],
                                        xt[k][:, 512 * n2:512 * (n2 + 1)],
                                        start=(k == 0), stop=(k == 5))
                                if m < 4:
                                    nc.vector.tensor_scalar_add(
                                        qkt[m][:, 512 * ng:512 * (ng + 1)], ps[:],
                                        bqk_sb[:, m:m + 1])
                                    continue
                                vt = ptp.tile([128, 512], f32r, name="vt", tag="pt")
                                nc.vector.tensor_scalar_add(
                                    vt[:], ps[:], bqk_sb[:, m:m + 1])
                                nheads = 2 if m == 4 else 1
                                for sub in range(4):
                                    gk = 4 * ng + sub
                                    tpv = ps_mm.tile([128, 128], f32r,
                                                     name="tpv", tag="mm")
                                    nc.tensor.transpose(
                                        tpv[:], vt[:, 128 * sub:128 * (sub + 1)],
                                        ident_sb[:])
                                    for i in range(nheads):
                                        hh = 2 * (m - 4) + i
                                        nc.vector.tensor_copy(
                                            vaug[hh][:, 65 * gk:65 * gk + 64],
                                            tpv[:, 64 * i:64 * (i + 1)])
                        # attention slices that just became ready (head 0)
                        att(0, 2 * v)
                        att(0, 2 * v + 1)
                        if v == 3:
                            allgather(0)

                # proj weights load under heads 1,2 compute
                for m in range(6):
                    nc.sync.dma_start(bpj_sb[:, m:m + 1],
                                      bproj[128 * m:128 * (m + 1), :])
                wpj = []
                for k in range(6):
                    wt = wpjp.tile([128, C], f32r, name=f"wpj{k}", tag=f"wpj{k}")
                    nc.sync.dma_start(wt[:], wproj[128 * k:128 * (k + 1), :])
                    wpj.append(wt)

                # ---- heads 1,2 (their compute covers the AllGathers) --------
                for j in range(NQS):
                    att(1, j)
                allgather(1)
                for j in range(NQS):
                    att(2, j)
                allgather(2)

            # ---------------- phase D: output projection ---------------------
            with tc.tile_pool(name="prhs", bufs=2) as prp, \
                 tc.tile_pool(name="pst", bufs=1) as pstp, \
                 tc.tile_pool(name="yst", bufs=2) as ystp:
                pT = [pstp.tile([128, TQ], f32r, name=f"pT{m}", tag=f"pT{m}")
                      for m in range(6)]
                for n2 in range(2):
                    rhs = []
                    for k in range(6):
                        rt = prp.tile([128, 512], f32r, name=f"rhs{k}", tag=f"rhs{k}")
                        row = 0
                        for src, srow, n in _ag_pieces(k):
                            nc.sync.dma_start(
                                rt[row:row + n, :],
                                ag_out[src][srow:srow + n,
                                            ds(qoff + 512 * n2, 512)])
                            row += n
                        rhs.append(rt)
                    korder = [0, 3, 1, 2, 4, 5]
                    for m in range(6):
                        ps = ps_mm.tile([128, 512], f32, name="psp", tag="mm")
                        for i, k in enumerate(korder):
                            nc.tensor.matmul(
                                ps[:], wpj[k][:, 128 * m:128 * (m + 1)], rhs[k][:],
                                start=(i == 0), stop=(i == 5))
                        nc.vector.tensor_scalar_add(
                            pT[m][:, 512 * n2:512 * (n2 + 1)], ps[:],
                            bpj_sb[:, m:m + 1])
                for t in range(8):
                    yt = ystp.tile([128, C], f32, name="yt", tag="yt")
                    for m in range(6):
                        tp = ps_mm.tile([128, 128], f32r, name="tpy", tag="mm")
                        nc.tensor.transpose(
                            tp[:], pT[m][:, 128 * t:128 * (t + 1)], ident_sb[:])
                        nc.vector.tensor_copy(yt[:, 128 * m:128 * (m + 1)], tp[:])
                    nc.sync.dma_start(yq[128 * t:128 * (t + 1), :], yt[:])

    nc.finalize()
    return nc


def _get_nc():
    if "nc" not in _nc_cache:
        _nc_cache["nc"] = _build()
    return _nc_cache["nc"]


def _host_inputs(x, W_qkv, b_qkv, W_proj, b_proj):
    x = np.ascontiguousarray(np.asarray(x, dtype=np.float32))
    W_qkv = np.asarray(W_qkv, dtype=np.float32)
    b_qkv = np.asarray(b_qkv, dtype=np.float32)
    W_proj = np.ascontiguousarray(np.asarray(W_proj, dtype=np.float32))
    b_proj = np.asarray(b_proj, dtype=np.float32)

    kk = np.arange(128)[:, None]
    qq = np.arange(512)[None, :]
    masks = np.empty((4, 128, 512), np.float32)
    for d in range(4):
        masks[d] = np.where(128 * d + kk <= qq, 0.0, NEG)
    ident = np.eye(128, dtype=np.float32)

    zpad = np.zeros((C, 64), np.float32)
    zb = np.zeros((64,), np.float32)

    in_maps = []
    for c in range(N_CORES):
        b, g = divmod(c, G)
        heads = [HPC * g + i for i in range(HPC)]
        qc = [W_qkv[:, 64 * h:64 * (h + 1)] for h in heads]
        kc = [W_qkv[:, C + 64 * h:C + 64 * (h + 1)] for h in heads]
        vc = [W_qkv[:, 2 * C + 64 * h:2 * C + 64 * (h + 1)] for h in heads]
        # cols: [Q0 Q1 | K0 K1 | Q2 pad | K2 pad | V0 V1 | V2 pad]
        wqkv_c = np.ascontiguousarray(np.concatenate(
            [qc[0], qc[1], kc[0], kc[1], qc[2], zpad, kc[2], zpad,
             vc[0], vc[1], vc[2], zpad], axis=1))
        bqh = [b_qkv[64 * h:64 * (h + 1)] for h in heads]
        bkh = [b_qkv[C + 64 * h:C + 64 * (h + 1)] for h in heads]
        bvh = [b_qkv[2 * C + 64 * h:2 * C + 64 * (h + 1)] for h in heads]
        bqk_c = np.concatenate(
            [bqh[0], bqh[1], bkh[0], bkh[1], bqh[2], zb, bkh[2], zb,
             bvh[0], bvh[1], bvh[2], zb])
        in_maps.append({
            "xb": x[b],
            "wqkv": wqkv_c,
            "bqk": np.ascontiguousarray(bqk_c.reshape(C, 1)),
            "wproj": W_proj,
            "bproj": np.ascontiguousarray(b_proj.reshape(C, 1)),
            "masks": masks,
            "ident": ident,
            "onesd": np.ones((128, 32), np.float32),
            "zerosd": np.zeros((128, 384), np.float32),
        })
    return in_maps


def kernel(x, W_qkv, b_qkv, W_proj, b_proj, _trace=False):
    import time
    from concourse.bass_utils import run_bass_kernel_spmd

    nc = _get_nc()
    in_maps = _host_inputs(x, W_qkv, b_qkv, W_proj, b_proj)
    last_err = None
    for attempt in range(3):
        try:
            res = run_bass_kernel_spmd(nc, in_maps, list(range(N_CORES)),
                                       trace=_trace)
            break
        except Exception as e:  # the axon terminal is transiently flaky
            last_err = e
            time.sleep(20)
    else:
        raise last_err
    y = np.empty((B, T, C), np.float32)
    for c in range(N_CORES):
        b, g = divmod(c, G)
        y[b, TQ * g:TQ * (g + 1), :] = res.results[c]["yq"]
    if _trace:
        kernel.last_results = res
    return y


# revision 25
# speedup vs baseline: 1.0163x; 1.0094x over previous
"""Causal self-attention block (qkv proj -> causal softmax attention -> out proj)
as a Bass/Tile SPMD kernel for 8 Trainium2 NeuronCores.

Sharding: data parallel over batch (B=2 -> 2 groups of 4 cores), tensor
parallel over heads within each group (12 heads -> 3 heads/core).  Each core:
  A. loads x[b], PE-transposes it to x^T in SBUF
  B. computes Q^T,K^T row-stacks and V (natural layout, augmented with a ones
     column per k-tile for the softmax denominator) for its 3 heads
  C. streaming causal attention per head without any P transposes:
       S^T tile = K_tile  Q^T          (matmul, lhsT = K^T slice)
       P^T = exp(scale*(S^T + mask))   (ScalarE, mask only on diagonal blocks)
       [O^T; sumexp] += [V|1]^T P^T    (matmul accumulate, 65-row psum)
       O^T *= 1/sumexp broadcast       (DVE recip + PE rank-1 broadcast)
     Attention emission is interleaved with phase B at q-slice granularity
     (slice j only needs x^T columns < 512(j+1)), so ScalarE exp work hides
     under phase B's PE work.  Heads 0,1 run interleaved; head 2 runs last so
     its compute covers the first two AllGathers.
  D. per-head AllGathers of O^T over the 4-core group, then the output
     projection for this core's quarter of the sequence (dynamic offset from
     partition id), PE-transposed back to natural layout.

All matmuls run in float32r (TF32-like, 1 cycle/row vs fp32's 4) with fp32
PSUM accumulation; measured matmul error ~1.5e-4 scale-relative.  Softmax
runs without max-subtraction (|scale*S| <= ~9 for this problem, exp is safe).

Matmul operands must share a base partition, so per-head 64-row Q^T/K^T
slices live in separate Q/K tensors at matching row offsets (heads 0,1 packed
in rows 0:64/64:128 of the Q and K stacks; head 2 in rows 0:64 of its own
pair, with 64 zero pad columns in the weight slice).
"""

import os
import sys

for _p in ("/opt/trn_rl_repo", "/root/.axon_site/_ro/trn_rl_repo"):
    if os.path.isdir(_p) and _p not in sys.path:
        sys.path.append(_p)

import numpy as np

B, T, C = 2, 4096, 768
H, DH = 12, 64
N_CORES = 8
G = 4                 # cores per batch group
HPC = 3               # heads per core
SC = HPC * DH         # 192: per-core width of each of Q/K/V
WC = 768              # weight cols: 128 each of Q01,K01,(Q2|pad),(K2|pad),V01,(V2|pad)
NQS = T // 512        # 8 q-slices of 512
NKT = T // 128        # 32 k-tiles of 128
QW = T // 4           # 1024: x processed in quarters
TQ = T // G           # 1024: per-core output rows
SCALE = 1.0 / np.sqrt(DH)
NEG = -1e30

_nc_cache = {}


def _ag_pieces(k):
    """Global O^T rows [128k,128k+128) as [(head, row, n)] pieces: the
    per-head AllGather h lays rank r's head-h rows at ag_out[h][64r:64r+64]."""
    out = []
    g = 128 * k
    while g < 128 * (k + 1):
        r, o192 = divmod(g, 192)
        i, o = divmod(o192, 64)
        out.append((i, 64 * r + o, 64))
        g += 64
    return out


def _build():
    import concourse.bass as bass
    import concourse.tile as tile
    import concourse.mybir as mybir
    from concourse import bacc
    from concourse.bass import ds

    f32 = mybir.dt.float32
    f32r = mybir.dt.float32r
    AF = mybir.ActivationFunctionType

    nc = bacc.Bacc(None, target_bir_lowering=False, debug=False, num_devices=N_CORES)

    xb = nc.dram_tensor("xb", [T, C], f32r, kind="ExternalInput")
    wqkv = nc.dram_tensor("wqkv", [C, WC], f32r, kind="ExternalInput")
    bqk = nc.dram_tensor("bqk", [768, 1], f32, kind="ExternalInput")
    wproj = nc.dram_tensor("wproj", [C, C], f32r, kind="ExternalInput")
    bproj = nc.dram_tensor("bproj", [C, 1], f32, kind="ExternalInput")
    masks = nc.dram_tensor("masks", [4, 128, 512], f32, kind="ExternalInput")
    ident = nc.dram_tensor("ident", [128, 128], f32r, kind="ExternalInput")
    onesd = nc.dram_tensor("onesd", [128, 32], f32r, kind="ExternalInput")
    zerosd = nc.dram_tensor("zerosd", [128, 384], f32r, kind="ExternalInput")

    ag_in = [nc.dram_tensor(f"ag_in{h}", [64, T], f32r) for h in range(HPC)]
    ag_out = [nc.dram_tensor(f"ag_out{h}", [4 * 64, T], f32r) for h in range(HPC)]
    yq = nc.dram_tensor("yq", [TQ, C], f32, kind="ExternalOutput")

    with tile.TileContext(nc) as tc:
        pid = nc.partition_id()
        qoff = (pid % G) * TQ

        with tc.tile_pool(name="const", bufs=1) as constp, \
             tc.tile_pool(name="wpj", bufs=1) as wpjp, \
             tc.tile_pool(name="psmm", bufs=6, space="PSUM") as ps_mm, \
             tc.tile_pool(name="psot", bufs=2, space="PSUM") as ps_ot:

            ident_sb = constp.tile([128, 128], f32r, name="ident_sb", tag="ident_sb")
            nc.sync.dma_start(ident_sb[:], ident[:])
            bqk_sb = constp.tile([128, 6], f32, name="bqk_sb", tag="bqk_sb")
            for m in range(6):
                nc.sync.dma_start(bqk_sb[:, m:m + 1], bqk[128 * m:128 * (m + 1), :])
            ones64 = constp.tile([1, 64], f32, name="ones64", tag="ones64")
            nc.vector.memset(ones64[:], 1.0)
            masks_sb = constp.tile([128, 4 * 512], f32, name="masks_sb", tag="masks_sb")
            bpj_sb = constp.tile([128, 6], f32, name="bpj_sb", tag="bpj_sb")
            zeros_sb = constp.tile([128, 384], f32r, name="zeros_sb", tag="zeros_sb")
            nc.sync.dma_start(zeros_sb[:], zerosd[:])

            with tc.tile_pool(name="qk", bufs=1) as qkp, \
                 tc.tile_pool(name="va", bufs=1) as vap, \
                 tc.tile_pool(name="pt", bufs=8) as ptp, \
                 tc.tile_pool(name="sm", bufs=3) as smp, \
                 tc.tile_pool(name="ost", bufs=3) as ostp:
                # qkt[0]=[Q0;Q1] qkt[1]=[K0;K1] qkt[2]=[Q2;pad] qkt[3]=[K2;pad]
                qkt = [qkp.tile([128, T], f32r, name=f"qkt{m}", tag=f"qkt{m}")
                       for m in range(4)]
                vaug = [vap.tile([128, NKT * 65], f32r, name=f"vaug{h}", tag=f"vaug{h}")
                        for h in range(HPC)]
                hq = [(qkt[0], 0), (qkt[0], 64), (qkt[2], 0)]
                hk = [(qkt[1], 0), (qkt[1], 64), (qkt[3], 0)]

                def att(h, j):
                    """One (head, 512-wide q-slice) causal attention block.
                    S-matmul + exp are emitted ahead of the PV matmuls so the
                    scheduler can run S(k+1..) on PE while ACT computes
                    exp(k), instead of stalling PE on each PV's input."""
                    qt_t, qt_r = hq[h]
                    kt_t, kt_r = hk[h]
                    otps = ps_ot.tile([65, 512], f32, name="otps", tag="ot")
                    last = 4 * j + 3
                    pts = []
                    for k0 in range(4 * j + 4):
                        sps = ps_mm.tile([128, 512], f32, name="sps", tag="mm")
                        nc.tensor.matmul(
                            sps[:],
                            kt_t[kt_r:kt_r + 64, 128 * k0:128 * (k0 + 1)],
                            qt_t[qt_r:qt_r + 64, 512 * j:512 * (j + 1)],
                            start=True, stop=True)
                        pt = ptp.tile([128, 512], f32r, name="pt", tag="pt")
                        if k0 // 4 == j:
                            # diagonal: cols < d0 are fully masked -> 0; add
                            # the causal mask and exp only on cols >= d0
                            d0 = 128 * (k0 % 4)
                            if d0 > 0:
                                nc.vector.tensor_copy(pt[:, 0:d0],
                                                      zeros_sb[:, 0:d0])
                            nc.vector.tensor_add(
                                pt[:, d0:512], sps[:, d0:512],
                                masks_sb[:, 512 * (k0 % 4) + d0:512 * (k0 % 4 + 1)])
                            nc.scalar.activation(
                                pt[:, d0:512], pt[:, d0:512], AF.Exp, scale=SCALE)
                        else:
                            nc.scalar.activation(pt[:], sps[:], AF.Exp, scale=SCALE)
                        pts.append(pt)
                    for k0 in range(4 * j + 4):
                        nc.tensor.matmul(
                            otps[:], vaug[h][:, 65 * k0:65 * k0 + 65], pts[k0][:],
                            start=(k0 == 0), stop=(k0 == last))
                    rc = smp.tile([1, 512], f32, name="rc", tag="rc")
                    nc.vector.reciprocal(rc[:], otps[64:65, :])
                    bc = ps_mm.tile([64, 512], f32, name="bc", tag="mm")
                    nc.tensor.matmul(bc[:], ones64[:], rc[:], start=True, stop=True)
                    bcs = smp.tile([64, 512], f32, name="bcs", tag="bcs")
                    nc.vector.tensor_copy(bcs[:], bc[:])
                    ost = ostp.tile([64, 512], f32r, name="ost", tag="ost")
                    nc.vector.tensor_mul(ost[:], otps[0:64, :], bcs[:])
                    nc.sync.dma_start(
                        ag_in[h][:, 512 * j:512 * (j + 1)], ost[:])

                def allgather(h):
                    nc.gpsimd.collective_compute(
                        "AllGather", mybir.AluOpType.bypass,
                        replica_groups=[[0, 1, 2, 3], [4, 5, 6, 7]],
                        ins=[ag_in[h][:]], outs=[ag_out[h][:]])

                # ---- phase A+B quarters, interleaved with heads 0,1 ----------
                with tc.tile_pool(name="wq", bufs=1) as wqp, \
                     tc.tile_pool(name="xn", bufs=4) as xnp, \
                     tc.tile_pool(name="xt", bufs=1) as xtp:
                    wq = []
                    for k in range(6):
                        wt = wqp.tile([128, WC], f32r, name=f"wq{k}", tag=f"wq{k}")
                        nc.sync.dma_start(wt[:], wqkv[128 * k:128 * (k + 1), :])
                        wq.append(wt)
                    for h in range(HPC):
                        nc.sync.dma_start(vaug[h][:, 64:NKT * 65:65], onesd[:])
                    for d in range(4):
                        nc.sync.dma_start(masks_sb[:, 512 * d:512 * (d + 1)], masks[d])

                    for v in range(4):
                        xt = [xtp.tile([128, QW], f32r, name=f"xt{k}", tag=f"xt{k}")
                              for k in range(6)]
                        for tt in range(8):
                            xn = xnp.tile([128, C], f32r, name="xn", tag="xn")
                            nc.scalar.dma_start(
                                xn[:], xb[QW * v + 128 * tt:QW * v + 128 * (tt + 1), :])
                            for k in range(6):
                                tp = ps_mm.tile([128, 128], f32r, name="tp", tag="mm")
                                nc.tensor.transpose(
                                    tp[:], xn[:, 128 * k:128 * (k + 1)], ident_sb[:])
                                nc.vector.tensor_copy(
                                    xt[k][:, 128 * tt:128 * (tt + 1)], tp[:])
                        # Q^T/K^T/V^T rows (6 m-tiles of 128); V^T m-tiles
                        # (4,5) are PE-transposed into natural-layout vaug
                        for n2 in range(2):
                            ng = 2 * v + n2
                            for m in range(6):
                                ps = ps_mm.tile([128, 512], f32, name="psb", tag="mm")
                                for k in range(6):
                                    nc.tensor.matmul(
                                        ps[:], wq[k][:, 128 * m:128 * (m + 1)],
                                        xt[k][:, 512 * n2:512 * (n2 + 1)],
                                        start=(k == 0), stop=(k == 5))
                                if m < 4:
                                    nc.vector.tensor_scalar_add(
                                        qkt[m][:, 512 * ng:512 * (ng + 1)], ps[:],
                                        bqk_sb[:, m:m + 1])
                                    continue
                                vt = ptp.tile([128, 512], f32r, name="vt", tag="pt")
                                nc.vector.tensor_scalar_add(
                                    vt[:], ps[:], bqk_sb[:, m:m + 1])
                                nheads = 2 if m == 4 else 1
                                for sub in range(4):
                                    gk = 4 * ng + sub
                                    tpv = ps_mm.tile([128, 128], f32r,
                                                     name="tpv", tag="mm")
                                    nc.tensor.transpose(
                                        tpv[:], vt[:, 128 * sub:128 * (sub + 1)],
                                        ident_sb[:])
                                    for i in range(nheads):
                                        hh = 2 * (m - 4) + i
                                        nc.vector.tensor_copy(
                                            vaug[hh][:, 65 * gk:65 * gk + 64],
                                            tpv[:, 64 * i:64 * (i + 1)])
                        # attention slices that just became ready (head 0)
                        att(0, 2 * v)
                        att(0, 2 * v + 1)
                        if v == 3:
                            allgather(0)

                # proj weights load under heads 1,2 compute
                for m in range(6):
                    nc.sync.dma_start(bpj_sb[:, m:m + 1],
                                      bproj[128 * m:128 * (m + 1), :])
                wpj = []
                for k in range(6):
                    wt = wpjp.tile([128, C], f32r, name=f"wpj{k}", tag=f"wpj{k}")
                    nc.sync.dma_start(wt[:], wproj[128 * k:128 * (k + 1), :])
                    wpj.append(wt)

                # ---- heads 1,2 (their compute covers the AllGathers) --------
                for j in range(NQS):
                    att(1, j)
                allgather(1)
                for j in range(NQS):
                    att(2, j)
                allgather(2)

            # ---------------- phase D: output projection ---------------------
            with tc.tile_pool(name="prhs", bufs=2) as prp, \
                 tc.tile_pool(name="pst", bufs=1) as pstp, \
                 tc.tile_pool(name="yst", bufs=2) as ystp:
                pT = [pstp.tile([128, TQ], f32r, name=f"pT{m}", tag=f"pT{m}")
                      for m in range(6)]
                for n2 in range(2):
                    rhs = []
                    for k in range(6):
                        rt = prp.tile([128, 512], f32r, name=f"rhs{k}", tag=f"rhs{k}")
                        row = 0
                        for src, srow, n in _ag_pieces(k):
                            nc.sync.dma_start(
                                rt[row:row + n, :],
                                ag_out[src][srow:srow + n,
                                            ds(qoff + 512 * n2, 512)])
                            row += n
                        rhs.append(rt)
                    korder = [0, 3, 1, 2, 4, 5]
                    for m in range(6):
                        ps = ps_mm.tile([128, 512], f32, name="psp", tag="mm")
                        for i, k in enumerate(korder):
                            nc.tensor.matmul(
                                ps[:], wpj[k][:, 128 * m:128 * (m + 1)], rhs[k][:],
                                start=(i == 0), stop=(i == 5))
                        nc.vector.tensor_scalar_add(
                            pT[m][:, 512 * n2:512 * (n2 + 1)], ps[:],
                            bpj_sb[:, m:m + 1])
                for t in range(8):
                    yt = ystp.tile([128, C], f32, name="yt", tag="yt")
                    for m in range(6):
                        tp = ps_mm.tile([128, 128], f32r, name="tpy", tag="mm")
                        nc.tensor.transpose(
                            tp[:], pT[m][:, 128 * t:128 * (t + 1)], ident_sb[:])
                        nc.vector.tensor_copy(yt[:, 128 * m:128 * (m + 1)], tp[:])
                    nc.sync.dma_start(yq[128 * t:128 * (t + 1), :], yt[:])

    nc.finalize()
    return nc


def _get_nc():
    if "nc" not in _nc_cache:
        _nc_cache["nc"] = _build()
    return _nc_cache["nc"]


def _host_inputs(x, W_qkv, b_qkv, W_proj, b_proj):
    x = np.ascontiguousarray(np.asarray(x, dtype=np.float32))
    W_qkv = np.asarray(W_qkv, dtype=np.float32)
    b_qkv = np.asarray(b_qkv, dtype=np.float32)
    W_proj = np.ascontiguousarray(np.asarray(W_proj, dtype=np.float32))
    b_proj = np.asarray(b_proj, dtype=np.float32)

    kk = np.arange(128)[:, None]
    qq = np.arange(512)[None, :]
    masks = np.empty((4, 128, 512), np.float32)
    for d in range(4):
        masks[d] = np.where(128 * d + kk <= qq, 0.0, NEG)
    ident = np.eye(128, dtype=np.float32)

    zpad = np.zeros((C, 64), np.float32)
    zb = np.zeros((64,), np.float32)

    in_maps = []
    for c in range(N_CORES):
        b, g = divmod(c, G)
        heads = [HPC * g + i for i in range(HPC)]
        qc = [W_qkv[:, 64 * h:64 * (h + 1)] for h in heads]
        kc = [W_qkv[:, C + 64 * h:C + 64 * (h + 1)] for h in heads]
        vc = [W_qkv[:, 2 * C + 64 * h:2 * C + 64 * (h + 1)] for h in heads]
        # cols: [Q0 Q1 | K0 K1 | Q2 pad | K2 pad | V0 V1 | V2 pad]
        wqkv_c = np.ascontiguousarray(np.concatenate(
            [qc[0], qc[1], kc[0], kc[1], qc[2], zpad, kc[2], zpad,
             vc[0], vc[1], vc[2], zpad], axis=1))
        bqh = [b_qkv[64 * h:64 * (h + 1)] for h in heads]
        bkh = [b_qkv[C + 64 * h:C + 64 * (h + 1)] for h in heads]
        bvh = [b_qkv[2 * C + 64 * h:2 * C + 64 * (h + 1)] for h in heads]
        bqk_c = np.concatenate(
            [bqh[0], bqh[1], bkh[0], bkh[1], bqh[2], zb, bkh[2], zb,
             bvh[0], bvh[1], bvh[2], zb])
        in_maps.append({
            "xb": x[b],
            "wqkv": wqkv_c,
            "bqk": np.ascontiguousarray(bqk_c.reshape(C, 1)),
            "wproj": W_proj,
            "bproj": np.ascontiguousarray(b_proj.reshape(C, 1)),
            "masks": masks,
            "ident": ident,
            "onesd": np.ones((128, 32), np.float32),
            "zerosd": np.zeros((128, 384), np.float32),
        })
    return in_maps


def kernel(x, W_qkv, b_qkv, W_proj, b_proj, _trace=False):
    import time
    from concourse.bass_utils import run_bass_kernel_spmd

    nc = _get_nc()
    in_maps = _host_inputs(x, W_qkv, b_qkv, W_proj, b_proj)
    last_err = None
    for attempt in range(3):
        try:
            res = run_bass_kernel_spmd(nc, in_maps, list(range(N_CORES)),
                                       trace=_trace)
            break
        except Exception as e:  # the axon terminal is transiently flaky
            last_err = e
            time.sleep(20)
    else:
        raise last_err
    y = np.empty((B, T, C), np.float32)
    for c in range(N_CORES):
        b, g = divmod(c, G)
        y[b, TQ * g:TQ * (g + 1), :] = res.results[c]["yq"]
    if _trace:
        kernel.last_results = res
    return y
